# revision 6
# baseline (speedup 1.0000x reference)
"""Grouped per-frequency linear (channels-first) for Trainium2, 8-core SPMD.

Reference op: y[b,o,t,f] = sum_c x[b,c,t,f] * W[f//U, f%U, c, o] + bias[o,0,f]
with B=8, C=64, T=1024, F=256, G=32, U=8, C_out=64 (fp32).

Sharding: data-parallel over batch B — core b processes x[b]; weight/bias
are tiny and replicated (host-packed into PE-friendly layouts).

Per-core kernel layout:
  * x tile in SBUF: 128 partitions = (f-half h, c_in c), free = (t, f_lo)
    so one matmul per frequency PAIR (f, f+128) uses the full 128x128 PE
    array with a block-diagonal stationary operand:
        rows 0:64   = W_f      (cols 0:64)
        rows 64:128 = W_{f+128} (cols 64:128)
  * matmul K=128, M=128, N=Tt (t-tile) -> PSUM partitions = (h, c_out)
  * DVE tensor_tensor add fuses bias-add with the PSUM->SBUF copy,
    writing the y tile in (h, c_out) x (t, f_lo) layout
  * y tile DMA'd back with the mirrored access pattern
"""

import numpy as np

B, C, T, F = 8, 64, 1024, 256
COUT = 64
H = 2           # frequency halves
FL = F // H     # 128 frequency pairs
TT = 64         # t-tile size
PB = 32         # pairs per PSUM block (PB*TT fp32 = 8KiB/partition = 4 banks)
N_CORES = 8

_CACHE = {}


def _build_nc(t_total: int):
    import concourse.tile as tile
    from concourse import bacc, mybir

    nt = t_total // TT
    nb = FL // PB

    nc = bacc.Bacc("TRN2", target_bir_lowering=False, debug=False,
                   num_devices=N_CORES)
    f32 = mybir.dt.float32
    x_d = nc.dram_tensor("x", [C, t_total, F], f32, kind="ExternalInput").ap()
    w_d = nc.dram_tensor("w", [H * C, FL * H * COUT], f32,
                         kind="ExternalInput").ap()
    b_d = nc.dram_tensor("bias", [H * COUT, FL], f32,
                         kind="ExternalInput").ap()
    y_d = nc.dram_tensor("y", [COUT, t_total, F], f32,
                         kind="ExternalOutput").ap()

    # DRAM views with dims ordered (h, chan, t, fl) to mirror the SBUF tiles
    x_v = x_d.rearrange("c t (h fl) -> h c t fl", h=H)
    y_v = y_d.rearrange("o t (h fl) -> h o t fl", h=H)

    with tile.TileContext(nc) as tc:
        with (
            tc.tile_pool(name="const", bufs=1) as const_pool,
            tc.tile_pool(name="xp", bufs=2) as xp,
            tc.tile_pool(name="yp", bufs=2) as yp,
            tc.tile_pool(name="ps", bufs=2, space="PSUM") as ps,
        ):
            w_t = const_pool.tile([H * C, FL * H * COUT], f32)
            nc.sync.dma_start(w_t[:], w_d[:])
            bias_t = const_pool.tile([H * COUT, FL], f32)
            nc.sync.dma_start(bias_t[:], b_d[:])

            for it in range(nt):
                t0 = it * TT
                xt = xp.tile([H * C, TT * FL], f32)
                for h in range(H):
                    nc.sync.dma_start(
                        xt[h * C:(h + 1) * C, :]
                            .rearrange("c (t fl) -> c t fl", fl=FL),
                        x_v[h, :, t0:t0 + TT, :],
                    )
                xt_v = xt[:].rearrange("q (t fl) -> q t fl", fl=FL)
                yt = yp.tile([H * COUT, TT * FL], f32)
                yt_v = yt[:].rearrange("q (t fl) -> q t fl", fl=FL)

                for ib in range(nb):
                    pt = ps.tile([H * COUT, PB * TT], f32)
                    pt_v = pt[:].rearrange("q (pl t) -> q pl t", t=TT)
                    for pl in range(PB):
                        p = ib * PB + pl
                        nc.tensor.matmul(
                            pt_v[:, pl, :],
                            w_t[:, p * H * COUT:(p + 1) * H * COUT],
                            xt_v[:, :, p],
                            start=True, stop=True,
                        )
                    nc.vector.tensor_add(
                        yt_v[:, :, ib * PB:(ib + 1) * PB].transpose([0, 2, 1]),
                        pt_v[:],
                        bias_t[:, ib * PB:(ib + 1) * PB].unsqueeze(2)
                              .broadcast_to([H * COUT, PB, TT]),
                    )

                for h in range(H):
                    nc.sync.dma_start(
                        y_v[h, :, t0:t0 + TT, :],
                        yt[h * COUT:(h + 1) * COUT, :]
                            .rearrange("o (t fl) -> o t fl", fl=FL),
                    )

    nc.compile()
    return nc


def _pack_inputs(weight, bias):
    """Host-side packing of the (tiny) weight/bias into device layouts."""
    wf = np.ascontiguousarray(weight, dtype=np.float32).reshape(F, C, COUT)
    w_blk = np.zeros((H * C, FL, H * COUT), np.float32)
    w_blk[0:C, :, 0:COUT] = wf[:FL].transpose(1, 0, 2)       # q=c, p, m=o
    w_blk[C:, :, COUT:] = wf[FL:].transpose(1, 0, 2)
    w_blk = w_blk.reshape(H * C, FL * H * COUT)

    b2 = np.asarray(bias, dtype=np.float32).reshape(COUT, F)
    bias_pk = np.concatenate([b2[:, :FL], b2[:, FL:]], axis=0)  # [128, 128]
    return np.ascontiguousarray(w_blk), np.ascontiguousarray(bias_pk)


def kernel(x, weight, bias, _trace=False, _return_res=False):
    from concourse.bass_utils import run_bass_kernel_spmd

    x = np.ascontiguousarray(x, dtype=np.float32)
    t_total = x.shape[2]
    if "nc" not in _CACHE or _CACHE.get("t_total") != t_total:
        _CACHE["nc"] = _build_nc(t_total)
        _CACHE["t_total"] = t_total
    nc = _CACHE["nc"]

    w_blk, bias_pk = _pack_inputs(weight, bias)
    in_maps = [{"x": x[b], "w": w_blk, "bias": bias_pk} for b in range(x.shape[0])]
    res = run_bass_kernel_spmd(nc, in_maps, list(range(x.shape[0])),
                               trace=_trace)
    y = np.stack([res.results[b]["y"] for b in range(x.shape[0])], axis=0)
    if _return_res:
        return y, res
    return y


# revision 7
# speedup vs baseline: 2.3453x; 2.3453x over previous
"""Grouped per-frequency linear (channels-first) for Trainium2, 8-core SPMD.

Reference op: y[b,o,t,f] = sum_c x[b,c,t,f] * W[f//U, f%U, c, o] + bias[o,0,f]
with B=8, C=64, T=1024, F=256, G=32, U=8, C_out=64 (fp32).

Sharding: data-parallel over batch B — core b processes x[b]; weight/bias
are tiny and replicated (host-packed into PE-friendly layouts). As part of
the shard prep, x is repacked per core to [2*C, T, F/2] (frequency halves
stacked on the channel axis) so that every device DMA is fully contiguous
per partition, and y is returned in the same packed layout and re-merged
on the host.

Per-core kernel:
  * x tile in SBUF: 128 partitions = (f-half h, c_in c), free = (t, f_lo).
  * One matmul per frequency PAIR (f, f+128) uses the full 128x128 PE
    array with a block-diagonal stationary operand:
        rows 0:64   = W_f       (cols 0:64)
        rows 64:128 = W_{f+128} (cols 64:128)
    K=128, M=128, N=Tt -> PSUM partitions = (h, c_out).
  * DVE tensor_tensor add fuses bias-add with the PSUM->SBUF copy.
  * y tile DMA'd out contiguously (x-in on the SP HWDGE ring, y-out on
    the ACT HWDGE ring so descriptor generation runs in parallel).
"""

import numpy as np

B, C, T, F = 8, 64, 1024, 256
COUT = 64
H = 2           # frequency halves
FL = F // H     # 128 frequency pairs
TT = 128        # t-tile size
PB = 32         # pairs per PSUM block (PB*TT fp32 = 16KiB... see build)
N_CORES = 8

_CACHE = {}


def _build_nc(t_total: int, tt: int = TT, pb: int = None, merged_xy: bool = True):
    import concourse.tile as tile
    from concourse import bacc, mybir

    if pb is None:
        # PSUM block: pb pairs * tt columns of fp32 <= 4 banks (2048 f32)
        pb = 2048 // tt
    nt = t_total // tt
    nb = FL // pb

    nc = bacc.Bacc("TRN2", target_bir_lowering=False, debug=False,
                   num_devices=N_CORES)
    f32 = mybir.dt.float32
    x_d = nc.dram_tensor("x", [H * C, t_total, FL], f32,
                         kind="ExternalInput").ap()
    w_d = nc.dram_tensor("w", [H * C, FL * H * COUT], f32,
                         kind="ExternalInput").ap()
    b_d = nc.dram_tensor("bias", [H * COUT, FL], f32,
                         kind="ExternalInput").ap()
    y_d = nc.dram_tensor("y", [H * COUT, t_total, FL], f32,
                         kind="ExternalOutput").ap()

    with tile.TileContext(nc) as tc:
        with (
            tc.tile_pool(name="const", bufs=1) as const_pool,
            tc.tile_pool(name="xp", bufs=2) as xp,
            tc.tile_pool(name="yp", bufs=2) as yp,
            tc.tile_pool(name="ps", bufs=2, space="PSUM") as ps,
        ):
            w_t = const_pool.tile([H * C, FL * H * COUT], f32)
            nc.gpsimd.dma_start(w_t[:], w_d[:])
            bias_t = const_pool.tile([H * COUT, FL], f32)
            nc.gpsimd.dma_start(bias_t[:], b_d[:])

            for it in range(nt):
                t0 = it * tt
                xt = xp.tile([H * C, tt * FL], f32)
                nc.sync.dma_start(
                    xt[:].rearrange("q (t fl) -> q t fl", fl=FL),
                    x_d[:, t0:t0 + tt, :],
                )
                xt_v = xt[:].rearrange("q (t fl) -> q t fl", fl=FL)
                if merged_xy:
                    yt = xt          # write y over the dead x columns
                    yt_v = xt_v
                else:
                    yt = yp.tile([H * COUT, tt * FL], f32)
                    yt_v = yt[:].rearrange("q (t fl) -> q t fl", fl=FL)

                for ib in range(nb):
                    pt = ps.tile([H * COUT, pb * tt], f32)
                    pt_v = pt[:].rearrange("q (pl t) -> q pl t", t=tt)
                    for pl in range(pb):
                        p = ib * pb + pl
                        nc.tensor.matmul(
                            pt_v[:, pl, :],
                            w_t[:, p * H * COUT:(p + 1) * H * COUT],
                            xt_v[:, :, p],
                            start=True, stop=True,
                        )
                    nc.vector.tensor_add(
                        yt_v[:, :, ib * pb:(ib + 1) * pb].transpose([0, 2, 1]),
                        pt_v[:],
                        bias_t[:, ib * pb:(ib + 1) * pb].unsqueeze(2)
                              .broadcast_to([H * COUT, pb, tt]),
                    )

                nc.scalar.dma_start(
                    y_d[:, t0:t0 + tt, :],
                    yt[:].rearrange("q (t fl) -> q t fl", fl=FL),
                )

    nc.compile()
    return nc


def _pack_weights(weight, bias):
    """Host-side packing of the (tiny) weight/bias into device layouts."""
    wf = np.ascontiguousarray(weight, dtype=np.float32).reshape(F, C, COUT)
    w_blk = np.zeros((H * C, FL, H * COUT), np.float32)
    w_blk[0:C, :, 0:COUT] = wf[:FL].transpose(1, 0, 2)       # q=c, p, m=o
    w_blk[C:, :, COUT:] = wf[FL:].transpose(1, 0, 2)
    w_blk = w_blk.reshape(H * C, FL * H * COUT)

    b2 = np.asarray(bias, dtype=np.float32).reshape(COUT, F)
    bias_pk = np.concatenate([b2[:, :FL], b2[:, FL:]], axis=0)  # [128, 128]
    return np.ascontiguousarray(w_blk), np.ascontiguousarray(bias_pk)


def _pack_x(x):
    # [B, C, T, F] -> [B, 2C, T, F/2]: stack the two frequency halves
    return np.ascontiguousarray(
        np.concatenate([x[:, :, :, :FL], x[:, :, :, FL:]], axis=1))


def _unpack_y(y_pk, n_b, t_total):
    # [B, 2*COUT, T, FL] -> [B, COUT, T, F]
    return np.ascontiguousarray(
        y_pk.reshape(n_b, H, COUT, t_total, FL)
            .transpose(0, 2, 3, 1, 4)
            .reshape(n_b, COUT, t_total, H * FL))


def kernel(x, weight, bias, _trace=False, _return_res=False):
    from concourse.bass_utils import run_bass_kernel_spmd

    x = np.asarray(x, dtype=np.float32)
    n_b, _, t_total, _ = x.shape
    if "nc" not in _CACHE or _CACHE.get("t_total") != t_total:
        _CACHE["nc"] = _build_nc(t_total)
        _CACHE["t_total"] = t_total
    nc = _CACHE["nc"]

    w_blk, bias_pk = _pack_weights(weight, bias)
    xp = _pack_x(x)
    in_maps = [{"x": xp[b], "w": w_blk, "bias": bias_pk} for b in range(n_b)]
    res = run_bass_kernel_spmd(nc, in_maps, list(range(n_b)), trace=_trace)
    y_pk = np.stack([res.results[b]["y"] for b in range(n_b)], axis=0)
    y = _unpack_y(y_pk, n_b, t_total)
    if _return_res:
        return y, res
    return y
